# revision 88
# baseline (speedup 1.0000x reference)
"""Trainium2 Bass kernel for nn_AttentionHAN (histogram_binning).

Strategy
--------
The reference network collapses algebraically (see the folding in
_fold_params): per batch row the device only needs 13 derived values
  sp(4)   pre-sigmoid attention scores
  tvd(4)  per-head dot of t_V with Wout[0,:128]
  ivd(4)  per-head dot of i_V with Wout[0,:128]
  base(1) [t_Q, i_Q] @ Wout[0,128:] + bout
plus the chi-square count statistics of t_V/i_V, which reduce to exact
per-feature counts S = #(v > thr) and C = #(v > thr and label==1).  With
s = sigmoid(sp), m1 = s*tvd, m2 = s*ivd, m3 = s*m2:
  out[b] = base + sum_h [ at*m1 + ai*m2 - (at*ai)*m3 ],
where at/ai = alpha_t/alpha_i depend on the GLOBAL chi statistics.

Performance design (vs the 127us fp32r baseline -> ~65.6us):
  - Everything on-device is float16: PE matmuls run at the same
    1 cycle/column as fp32r while input DMA bytes halve (32MB -> 16MB per
    core).  End-to-end rel err vs the fp32 reference is ~4.8e-4.
  - Launch A streams x in [128, 1024]-column superblocks (two 512-row
    tail superblocks shorten the drain) (360GB/s DMA is
    the roofline: ~46.6us/core; the x DMAs are issued ahead of the packed
    constants so the stream starts at the earliest slot, and R flushes ride
    the GPSIMD software-DGE path so the SP/ACT trigger pipelines never
    stall the stream -- the DMA engine runs gapless) and per 512-row block
    computes
      PE:  t_V/i_V (two K=128 fp16 matmuls each, N=512) and the 13 R values
           via transposed matmuls (lhsT = 128-row batch slice of x,
           rhs = folded (128,13) weight chunk, N=13 -> 13 cycles each);
           one K=1 N=52 matmul seeds the PSUM accumulator with the bias.
      DVE: binarize+count for both modalities (is_gt vs per-partition
           threshold with accumulator -> per-block S counts).
      ACT: sigmoid(sp) and the PSUM->SBUF fp16 copy of tvd/ivd/base.
      GPSIMD: the m1/m2/m3 products in place in the staging tile.
    Every engine stays under the 1456ns/block DMA streaming rate.
  - The single possibly-mixed label block is rotated to block 0 so its
    label-weighted count (C) ops overlap the stream instead of the tail.
  - Counts are reduced on host (the tiny "all-reduce" of the sharding
    hint) and alpha is computed exactly as the reference does.
  - Launch B applies the per-row combination as one broadcast fp16
    multiply (rb * [at, ai, -at*ai, 1], stride-0 AP, DVE 2x mode) plus a
    segmented 13:1 reduce, pipelined in three ascending chunks whose DMA
    triggers alternate between the SP and ACT queues; the coefficient
    vector rides as a 13-column prefix of the first rb transfer.

Sharding: pure data parallel over B on 8 cores (16384 rows each), params
replicated; host gathers/unpermutes the final (B, 1) fp32 output.
"""

import sys
import numpy as np

sys.path.insert(0, "/opt/trn_rl_repo")

import concourse.bacc as bacc  # noqa: E402
import concourse.tile as tile  # noqa: E402
from concourse import mybir  # noqa: E402
from concourse.bass import broadcast_tensor_aps  # noqa: E402

F16 = mybir.dt.float16
F32 = mybir.dt.float32
f16 = np.float16
f32 = np.float32

B_TOT = 131072
IN = 256
HID = 128
H = 4
D = 32
NCORES = 8
THRESH = 0.7
BLK = 512
RPC = B_TOT // NCORES          # 16384 rows per core
NBLK = RPC // BLK              # 32 blocks of 512
SBCOLS = 1024                  # max superblock width (x-tile DMA granularity)
SUPER = [1024] * 15 + [512, 512]  # superblock sizes
XBUFS = 16                     # x-tile buffering depth
FLUSH = 8                      # blocks per R-staging flush

# cf16 packed-constant column offsets
_WTV0, _WTV1 = 0, 128
_WIV0, _WIV1 = 256, 384
_WSMT0, _WSMT1 = 512, 525
_WSMI0, _WSMI1 = 538, 551
_BSM = 564                     # bsm tiled x4 (52 cols)
_ONES = 616
_LAB = 744
_CF16 = 744 + BLK

_cache = {}


def _build_kernel_a():
    nc = bacc.Bacc("TRN2", target_bir_lowering=False, debug=False)
    xt = nc.dram_tensor("xt", (IN, RPC), F16, kind="ExternalInput")
    xi = nc.dram_tensor("xi", (IN, RPC), F16, kind="ExternalInput")
    cf16 = nc.dram_tensor("cf16", (128, _CF16), F16, kind="ExternalInput")
    cf32 = nc.dram_tensor("cf32", (128, 2), F32, kind="ExternalInput")
    r_out = nc.dram_tensor("r_out", (128, 52 * NBLK), F16,
                           kind="ExternalOutput")
    aux_out = nc.dram_tensor("aux_out", (128, 2 * NBLK + 2), F32,
                             kind="ExternalOutput")

    with tile.TileContext(nc) as tc:
        with (
            tc.tile_pool(name="w", bufs=1) as wp,
            tc.tile_pool(name="x", bufs=XBUFS) as xp,
            tc.tile_pool(name="fv", bufs=2) as fp,
            tc.tile_pool(name="acc", bufs=1) as ap,
            tc.tile_pool(name="rt", bufs=4) as rp,
            tc.tile_pool(name="ptv", bufs=3, space="PSUM") as ptvp,
            tc.tile_pool(name="piv", bufs=3, space="PSUM") as pivp,
            tc.tile_pool(name="pr", bufs=2, space="PSUM") as prp,
        ):
            cf = wp.tile([128, _CF16], F16, tag="cf16")
            cw = wp.tile([128, 2], F32, tag="cf32")
            wtv_sb = [cf[:, _WTV0:_WTV0 + 128], cf[:, _WTV1:_WTV1 + 128]]
            wiv_sb = [cf[:, _WIV0:_WIV0 + 128], cf[:, _WIV1:_WIV1 + 128]]
            wsmt_sb = [cf[:, _WSMT0:_WSMT0 + 13], cf[:, _WSMT1:_WSMT1 + 13]]
            wsmi_sb = [cf[:, _WSMI0:_WSMI0 + 13], cf[:, _WSMI1:_WSMI1 + 13]]
            bsm52_sb = cf[0:1, _BSM:_BSM + 52]
            ones_sb = cf[0:1, _ONES:_ONES + 128]
            lab_sb = cf[0:1, _LAB:_LAB + BLK]
            thrt_sb = cw[:, 0:1]
            thri_sb = cw[:, 1:2]

            aux_sb = ap.tile([128, 2 * NBLK + 2], F32, tag="aux")

            blk = 0
            rt = None
            rts = {}

            def emit_mm_tviv(b, x4, o):
                ptv = ptvp.tile([128, BLK], F32, name="ptv", tag="ptv")
                piv = pivp.tile([128, BLK], F32, name="piv", tag="piv")
                nc.tensor.matmul(ptv[:], wtv_sb[0], x4[0][:, o:o + BLK],
                                 start=True, stop=False)
                nc.tensor.matmul(ptv[:], wtv_sb[1], x4[1][:, o:o + BLK],
                                 start=False, stop=True)
                nc.tensor.matmul(piv[:], wiv_sb[0], x4[2][:, o:o + BLK],
                                 start=True, stop=False)
                nc.tensor.matmul(piv[:], wiv_sb[1], x4[3][:, o:o + BLK],
                                 start=False, stop=True)
                return ptv, piv

            def emit_mm_sm(b, x4, o):
                pr = prp.tile([128, 52], F32, name="pr", tag="pr")
                nc.tensor.matmul(pr[:], ones_sb, bsm52_sb,
                                 start=True, stop=False,
                                 skip_group_check=True)
                for c in range(4):
                    sl = pr[:, c * 13:(c + 1) * 13]
                    oc = o + c * 128
                    nc.tensor.matmul(sl, x4[0][:, oc:oc + 128], wsmt_sb[0],
                                     start=False, stop=False,
                                     skip_group_check=True)
                    nc.tensor.matmul(sl, x4[1][:, oc:oc + 128], wsmt_sb[1],
                                     start=False, stop=False,
                                     skip_group_check=True)
                    nc.tensor.matmul(sl, x4[2][:, oc:oc + 128], wsmi_sb[0],
                                     start=False, stop=False,
                                     skip_group_check=True)
                    nc.tensor.matmul(sl, x4[3][:, oc:oc + 128], wsmi_sb[1],
                                     start=False, stop=(c == 3),
                                     skip_group_check=True)
                return pr

            def emit_counts(b, ptv, piv):
                # binarize + S count for both modalities (DVE)
                fvt = fp.tile([128, BLK], F16, tag="fvt")
                nc.vector.tensor_scalar(
                    fvt[:], ptv[:], thrt_sb, None,
                    op0=mybir.AluOpType.is_gt, op1=mybir.AluOpType.add,
                    accum_out=aux_sb[:, b:b + 1])
                fvi = fp.tile([128, BLK], F16, tag="fvi")
                nc.vector.tensor_scalar(
                    fvi[:], piv[:], thri_sb, None,
                    op0=mybir.AluOpType.is_gt, op1=mybir.AluOpType.add,
                    accum_out=aux_sb[:, NBLK + b:NBLK + b + 1])
                if b == 0:
                    # the single possibly-mixed block (rotated to the front):
                    # label-weighted counts.  Label row broadcast across
                    # partitions via a K=1 matmul.
                    plab = ptvp.tile([128, BLK], F32, name="plab", tag="ptv")
                    nc.tensor.matmul(plab[:], ones_sb, lab_sb,
                                     start=True, stop=True)
                    fvl = fp.tile([128, BLK], F16, tag="fvl")
                    nc.vector.scalar_tensor_tensor(
                        fvl[:], fvt[:], 1.0, plab[:],
                        op0=mybir.AluOpType.mult, op1=mybir.AluOpType.mult,
                        accum_out=aux_sb[:, 2 * NBLK:2 * NBLK + 1])
                    nc.vector.scalar_tensor_tensor(
                        fvl[:], fvi[:], 1.0, plab[:],
                        op0=mybir.AluOpType.mult, op1=mybir.AluOpType.mult,
                        accum_out=aux_sb[:, 2 * NBLK + 1:2 * NBLK + 2])

            def emit_r(b, pr):
                # R: s = sigmoid(sp) [ACT]; copy r-values; m1/m2/m3
                pr3 = pr[:].rearrange("p (g k) -> p g k", k=13)
                s = fp.tile([128, 16], F16, tag="s")
                s3 = s[:].rearrange("p (g k) -> p g k", k=4)
                nc.scalar.activation(s3, pr3[:, :, 0:4],
                                     mybir.ActivationFunctionType.Sigmoid)
                rt3 = rts[b // FLUSH][:, (b % FLUSH) * 52:
                                      (b % FLUSH + 1) * 52] \
                    .rearrange("p (g k) -> p g k", k=13)
                nc.scalar.activation(rt3[:, :, 4:13], pr3[:, :, 4:13],
                                     mybir.ActivationFunctionType.Copy)
                # final block's m-products on DVE (it drains earlier than
                # the Pool queue with the 512-row tail superblocks)
                eng = nc.vector if b == NBLK - 1 else nc.gpsimd
                eng.tensor_tensor(rt3[:, :, 0:4], s3,
                                  rt3[:, :, 4:8],
                                  op=mybir.AluOpType.mult)
                eng.tensor_tensor(rt3[:, :, 4:8], s3,
                                  rt3[:, :, 8:12],
                                  op=mybir.AluOpType.mult)
                eng.tensor_tensor(rt3[:, :, 8:12], s3,
                                  rt3[:, :, 4:8],
                                  op=mybir.AluOpType.mult)

            def emit_flush(b):
                g = b // FLUSH
                if b == NBLK - 2:
                    # pre-flush the last group's first 7 blocks (GPSIMD) so
                    # the final critical-path DMA carries only block 31
                    nc.gpsimd.dma_start(
                        r_out[:, g * 52 * FLUSH:g * 52 * FLUSH + 52 * 7],
                        rts[g][:, 0:52 * 7])
                elif b == NBLK - 1:
                    # aux first (its wait clears earliest), then the final
                    # 52-col flush -- both on SP so the DGE chains pipeline
                    nc.sync.dma_start(aux_out[:], aux_sb[:])
                    nc.sync.dma_start(
                        r_out[:, (g + 1) * 52 * FLUSH - 52:
                               (g + 1) * 52 * FLUSH],
                        rts[g][:, 52 * 7:])
                elif b % FLUSH == FLUSH - 1:
                    # mid-stream flushes ride the GPSIMD SWDGE path (keeps
                    # the SP/ACT trigger pipelines clear of the x stream)
                    nc.gpsimd.dma_start(
                        r_out[:, g * 52 * FLUSH:(g + 1) * 52 * FLUSH],
                        rts[g][:])

            for sb, size in enumerate(SUPER):
                off = blk * BLK
                xt0 = xp.tile([128, SBCOLS], F16, tag="xt0")
                xt1 = xp.tile([128, SBCOLS], F16, tag="xt1")
                xi0 = xp.tile([128, SBCOLS], F16, tag="xi0")
                xi1 = xp.tile([128, SBCOLS], F16, tag="xi1")
                x4 = (xt0, xt1, xi0, xi1)
                nc.sync.dma_start(xt0[:, :size], xt[0:128, off:off + size])
                nc.sync.dma_start(xt1[:, :size], xt[128:256, off:off + size])
                nc.sync.dma_start(xi0[:, :size], xi[0:128, off:off + size])
                nc.sync.dma_start(xi1[:, :size], xi[128:256, off:off + size])
                if sb == 0:
                    # consts follow the first x superblock so the big input
                    # stream starts at the earliest possible DMA slot
                    nc.sync.dma_start(cf[:], cf16[:])
                    nc.sync.dma_start(cw[:], cf32[:])
                nblk_sb = size // BLK
                for j in range(nblk_sb):
                    if (blk + j) % FLUSH == 0:
                        rts[(blk + j) // FLUSH] = rp.tile(
                            [128, 52 * FLUSH], F16,
                            name=f"rt{(blk + j) // FLUSH}", tag="rt")
                for j in range(nblk_sb):
                    o = j * BLK
                    ptv, piv = emit_mm_tviv(blk, x4, o)
                    pr = emit_mm_sm(blk, x4, o)
                    emit_counts(blk, ptv, piv)
                    emit_r(blk, pr)
                    emit_flush(blk)
                    blk += 1

    nc.compile()
    return nc


def _build_kernel_b():
    from concourse.bass import broadcast_tensor_aps

    nc = bacc.Bacc("TRN2", target_bir_lowering=False, debug=False)
    # rb = [w13(13) | r values(52*NBLK)]: the coefficient vector rides in
    # front of the first chunk's transfer instead of its own DMA
    rb = nc.dram_tensor("rb", (128, 13 + 52 * NBLK), F16,
                        kind="ExternalInput")
    o_out = nc.dram_tensor("o_out", (128, 4 * NBLK), F16,
                           kind="ExternalOutput")

    nch = 4 * NBLK
    bounds = [0, 40, 72, nch]     # ascending chunks: small first so the DVE
                                  # chain starts while the rest streams in
                                  # (19 keeps chunk0 >= 512B/partition)
    with tile.TileContext(nc) as tc:
        with tc.tile_pool(name="s", bufs=1) as sp:
            rb_sb = sp.tile([128, 13 + 52 * NBLK], F16, tag="rb")
            mm = sp.tile([128, 52 * NBLK], F16, tag="mm")
            out_sb = sp.tile([128, nch], F16, tag="o")
            w3 = rb_sb[:, 0:13].rearrange("p (c k) -> p c k", k=13)
            # alternate ACT/SP trigger queues per chunk -- independent
            # trigger pipelines so the first rb bytes land early
            engs = [nc.sync, nc.scalar, nc.sync]
            for ch in range(len(bounds) - 1):
                c0, c1 = 13 + bounds[ch] * 13, 13 + bounds[ch + 1] * 13
                if ch == 0:
                    c0 = 0    # chunk 0 carries the w13 prefix
                engs[ch].dma_start(rb_sb[:, c0:c1], rb[:, c0:c1])
            for ch in range(len(bounds) - 1):
                c0, c1 = bounds[ch], bounds[ch + 1]
                rb3 = rb_sb[:, 13 + c0 * 13:13 + c1 * 13].rearrange(
                    "p (c k) -> p c k", k=13)
                mm3 = mm[:, c0 * 13:c1 * 13].rearrange(
                    "p (c k) -> p c k", k=13)
                rb3b, w3b = broadcast_tensor_aps(rb3, w3)
                nc.vector.tensor_tensor(mm3, rb3b, w3b,
                                        op=mybir.AluOpType.mult)
                with nc.allow_low_precision(reason="13-term fp16 row reduce"):
                    nc.vector.tensor_reduce(
                        out_sb[:, c0:c1], mm3,
                        axis=mybir.AxisListType.X, op=mybir.AluOpType.add)
                nc.sync.dma_start(o_out[:, c0:c1], out_sb[:, c0:c1])

    nc.compile()
    return nc


def _get_kernels():
    if "a" not in _cache:
        _cache["a"] = _build_kernel_a()
        _cache["b"] = _build_kernel_b()
    return _cache["a"], _cache["b"]


class _Runner:
    """Persistent jitted SPMD executor for a compiled Bass module.

    Mirrors bass2jax.run_bass_via_pjrt but keeps the jitted callable alive so
    repeated kernel() invocations skip retracing/recompilation."""

    def __init__(self, nc):
        import jax
        from jax.sharding import Mesh, PartitionSpec
        from jax.experimental.shard_map import shard_map
        from concourse import bass2jax

        bass2jax.install_neuronx_cc_hook()
        self._nc = nc
        pname = nc.partition_id_tensor.name if nc.partition_id_tensor else None
        in_names, out_names, out_avals = [], [], []
        self._zero_outs = []
        for alloc in nc.m.functions[0].allocations:
            if not isinstance(alloc, mybir.MemoryLocationSet):
                continue
            nm = alloc.memorylocations[0].name
            if alloc.kind == "ExternalInput":
                if nm != pname:
                    in_names.append(nm)
            elif alloc.kind == "ExternalOutput":
                out_names.append(nm)
                shape = tuple(alloc.tensor_shape)
                dt = mybir.dt.np(alloc.dtype)
                out_avals.append(jax.core.ShapedArray(shape, dt))
                self._zero_outs.append(np.zeros(shape, dt))
        self._in_names = in_names
        self._out_names = out_names
        all_in_names = in_names + out_names + ([pname] if pname else [])

        def _body(*args):
            operands = list(args)
            if pname:
                operands.append(bass2jax.partition_id_tensor())
            outs = bass2jax._bass_exec_p.bind(
                *operands, out_avals=tuple(out_avals),
                in_names=tuple(all_in_names), out_names=tuple(out_names),
                lowering_input_output_aliases=(), sim_require_finite=True,
                sim_require_nnan=True, nc=nc)
            return tuple(outs)

        devices = jax.devices()[:NCORES]
        assert len(devices) == NCORES, f"need {NCORES} devices"
        mesh = Mesh(np.asarray(devices), ("core",))
        nio = len(in_names) + len(out_names)
        self._fn = jax.jit(
            shard_map(_body, mesh=mesh,
                      in_specs=(PartitionSpec("core"),) * nio,
                      out_specs=(PartitionSpec("core"),) * len(out_names),
                      check_rep=False),
            keep_unused=True)

    def __call__(self, in_maps):
        assert len(in_maps) == NCORES
        concat = [
            np.concatenate([np.asarray(m[n]) for m in in_maps], axis=0)
            for n in self._in_names
        ]
        concat += [
            np.zeros((NCORES * z.shape[0], *z.shape[1:]), z.dtype)
            for z in self._zero_outs
        ]
        out_arrs = self._fn(*concat)
        results = []
        for c in range(NCORES):
            d = {}
            for i, nm in enumerate(self._out_names):
                full = np.asarray(out_arrs[i])
                per = full.shape[0] // NCORES
                d[nm] = full[c * per:(c + 1) * per]
            results.append(d)
        return results


def _get_runners():
    if "ra" not in _cache:
        nc_a, nc_b = _get_kernels()
        _cache["ra"] = _Runner(nc_a)
        _cache["rb"] = _Runner(nc_b)
    return _cache["ra"], _cache["rb"]


def _fold_params(p):
    """Fold all network params into the device weight matrices (host, f64)."""
    Wout = p["Wout"].astype(np.float64)
    bout = p["bout"].astype(np.float64)
    attn_W = p["attn_W"].astype(np.float64)
    attn_b = p["attn_b"].astype(np.float64)
    W1 = Wout[0, :HID]          # fused part
    W2 = Wout[0, HID:2 * HID]   # t_Q part
    W3 = Wout[0, 2 * HID:]      # i_Q part

    # A_t[32h+d, h] = attn_W[h, d];  A_i[32h+d, h] = attn_W[h, 32+d]
    A_t = np.zeros((HID, H))
    A_i = np.zeros((HID, H))
    Bt = np.zeros((HID, H))
    for h in range(H):
        A_t[h * D:(h + 1) * D, h] = attn_W[h, :D]
        A_i[h * D:(h + 1) * D, h] = attn_W[h, D:]
        Bt[h * D:(h + 1) * D, h] = W1[h * D:(h + 1) * D]

    def WT(name):
        return p[name].astype(np.float64).T  # (IN, HID)

    wsmt = np.zeros((IN, 13))
    wsmt[:, 0:4] = WT("Wtq") @ A_t
    wsmt[:, 4:8] = WT("Wtv") @ Bt
    wsmt[:, 12] = WT("Wtq") @ W2
    wsmi = np.zeros((IN, 13))
    wsmi[:, 0:4] = WT("Wik") @ A_i
    wsmi[:, 8:12] = WT("Wiv") @ Bt
    wsmi[:, 12] = WT("Wiq") @ W3

    bsm = np.zeros(13)
    bsm[0:4] = (p["btq"].astype(np.float64) @ A_t
                + p["bik"].astype(np.float64) @ A_i + attn_b)
    bsm[4:8] = p["btv"].astype(np.float64) @ Bt
    bsm[8:12] = p["biv"].astype(np.float64) @ Bt
    bsm[12] = (p["btq"].astype(np.float64) @ W2
               + p["biq"].astype(np.float64) @ W3 + bout[0])

    cf32 = np.zeros((128, 2), dtype=f32)
    cf32[:, 0] = f32(THRESH) - p["btv"].astype(f32)   # t threshold
    cf32[:, 1] = f32(THRESH) - p["biv"].astype(f32)   # i threshold
    return {
        "wtv": WT("Wtv").astype(f16),     # (256, 128)
        "wiv": WT("Wiv").astype(f16),
        "wsmt": wsmt.astype(f16),         # (256, 13)
        "wsmi": wsmi.astype(f16),
        "bsm": bsm.astype(f16),           # (13,)
        "cf32": cf32,
    }


def _build_cf16(folded, lab_row):
    cf = np.zeros((128, _CF16), dtype=f16)
    cf[:, _WTV0:_WTV0 + 128] = folded["wtv"][0:128]
    cf[:, _WTV1:_WTV1 + 128] = folded["wtv"][128:256]
    cf[:, _WIV0:_WIV0 + 128] = folded["wiv"][0:128]
    cf[:, _WIV1:_WIV1 + 128] = folded["wiv"][128:256]
    cf[:, _WSMT0:_WSMT0 + 13] = folded["wsmt"][0:128]
    cf[:, _WSMT1:_WSMT1 + 13] = folded["wsmt"][128:256]
    cf[:, _WSMI0:_WSMI0 + 13] = folded["wsmi"][0:128]
    cf[:, _WSMI1:_WSMI1 + 13] = folded["wsmi"][128:256]
    cf[0, _BSM:_BSM + 52] = np.tile(folded["bsm"], 4)
    cf[0, _ONES:_ONES + 128] = f16(1.0)
    cf[0, _LAB:_LAB + BLK] = lab_row
    return cf


def _chi_square_from_counts(S, C, L, B):
    """Replicate the reference chi-square given exact integer counts (f32 ops)."""
    F = S.shape[0]
    counts = np.zeros((F, 2, 2), dtype=f32)
    counts[:, 1, 1] = C
    counts[:, 1, 0] = S - C
    counts[:, 0, 1] = L - C
    counts[:, 0, 0] = B - S - L + C
    total = counts.sum(axis=(1, 2), dtype=f32)
    col = counts.sum(axis=1, dtype=f32)   # (F,2) over f_val -> label counts
    row = counts.sum(axis=2, dtype=f32)   # (F,2) over l_val -> feature counts
    expected = col[:, :, None] * row[:, None, :] / (total[:, None, None] + f32(1e-6))
    chi = ((counts - expected) ** 2 / (expected + f32(1e-6))).sum(
        axis=(1, 2), dtype=f32)
    return chi


def kernel(**inputs):
    text = np.asarray(inputs["text_vec"], dtype=f32).astype(f16)
    image = np.asarray(inputs["image_vec"], dtype=f32).astype(f16)
    label = np.asarray(inputs["label"]).astype(np.int64)

    folded = _fold_params(inputs)
    run_a, run_b = _get_runners()

    # Row assignment: sort all rows by label, deal contiguous RPC-row chunks
    # to cores, then within each core rotate the (at most one) mixed 512-row
    # block to device block index 0, so blocks 1..31 are label-pure and only
    # block 0 needs the on-device label-weighted count (overlapped with the
    # stream instead of extending the tail).
    order = np.concatenate([np.flatnonzero(label == 0),
                            np.flatnonzero(label != 0)])
    in_maps = []
    srcs = []
    pure1_masks = []
    l0s = []
    for c in range(NCORES):
        chunk = order[c * RPC:(c + 1) * RPC]
        n0 = int((label[chunk] == 0).sum())
        k0, r0 = divmod(n0, BLK)
        if r0 > 0:
            src = np.concatenate([chunk[k0 * BLK:(k0 + 1) * BLK],
                                  chunk[0:k0 * BLK],
                                  chunk[(k0 + 1) * BLK:]])
        else:
            src = chunk
        lab_perm = (label[src] != 0)
        blocks = lab_perm.reshape(NBLK, BLK)
        pure1 = blocks.all(axis=1)
        mixed = blocks.any(axis=1) & ~pure1
        assert not mixed[1:].any(), "mixed block must be at index 0"
        m = {
            "xt": np.ascontiguousarray(text[src].T),
            "xi": np.ascontiguousarray(image[src].T),
            "cf16": _build_cf16(folded, lab_perm[:BLK].astype(f16)),
            "cf32": folded["cf32"],
        }
        in_maps.append(m)
        srcs.append(src)
        pure1_masks.append(pure1[1:])
        l0s.append(float(lab_perm[:BLK].sum()))

    # ---- launch A
    res_a = run_a(in_maps)

    # ---- host: reduce the tiny count tables, compute alpha (the "all-reduce")
    S_t = np.zeros(HID)
    S_i = np.zeros(HID)
    C_t = np.zeros(HID)
    C_i = np.zeros(HID)
    for c in range(NCORES):
        aux = res_a[c]["aux_out"].astype(np.float64)
        st = aux[:, 0:NBLK]
        si = aux[:, NBLK:2 * NBLK]
        S_t += st.sum(axis=1)
        S_i += si.sum(axis=1)
        p1 = pure1_masks[c]
        C_t += st[:, 1:][:, p1].sum(axis=1) + aux[:, 2 * NBLK]
        C_i += si[:, 1:][:, p1].sum(axis=1) + aux[:, 2 * NBLK + 1]
    L = float((label != 0).sum())
    chi_t = _chi_square_from_counts(S_t, C_t, L, float(B_TOT))
    chi_i = _chi_square_from_counts(S_i, C_i, L, float(B_TOT))
    chi_max = f32(max(chi_t.max(), chi_i.max()))
    alpha_t = (chi_t / (chi_max + f32(1e-6)))[:H].astype(f32)
    alpha_i = (chi_i / (chi_max + f32(1e-6)))[:H].astype(f32)

    w13 = np.concatenate([alpha_t, alpha_i, -(alpha_t * alpha_i),
                          [f32(1.0)]]).astype(f16)
    w13_t = np.ascontiguousarray(
        np.broadcast_to(w13[None, :], (128, 13)))

    in_maps_b = [{"rb": np.hstack([w13_t, res_a[c]["r_out"]])}
                 for c in range(NCORES)]

    # ---- launch B
    res_b = run_b(in_maps_b)

    # ---- gather (undo the per-core row permutation)
    out = np.empty((B_TOT, 1), dtype=f32)
    for c in range(NCORES):
        o = res_b[c]["o_out"].astype(f32)  # (128, 128); row = col*128 + p
        rows = o.T.reshape(RPC)
        out[srcs[c], 0] = rows
    return out
